# revision 30
# baseline (speedup 1.0000x reference)
"""Trainium2 Bass kernel for nn_DechunkingLayer (ragged_sequence).

Reference semantics (per batch row):
    idx = clip(exclusive_cumsum(b), 0, NC - 1)          # [T]
    up[t]  = z[idx[t]]                                  # gather rows
    out[t] = p[t] * up[t] + (1 - p[t]) * up[t-1]        # EMA blend
    out[0] = up[0]

Sharding: pure data parallel over batch B=8 across the 8 NeuronCores
(one batch row per core). All work per row is independent.

Per-core plan. HBM traffic = 16 MB gather + 16 MB store = 32 MB and the
steady state runs at the HBM roofline (~410 GB/s combined), so the
optimization targets are the two ends of the timeline:
  - constants (triangular scan matrix, shifted identity, ...) come in as
    host-provided input tensors, split hot/cold across the two HWDGE
    rings so the scan matrix lands ~2 us after the NEFF preamble; no
    gpsimd constant building, no PE warm-up matmuls ahead of the scan.
  - tile-0 gather indices take a short path: b[0:128] is DMA'd as a
    [128, 1] column directly (colofs[0] = 0, so column 0 needs only the
    partition-dim triangular-matmul scan).
  - full cumsum in the [128, 32] "W layout" (partition = t % 128,
    column = t // 128) via PE triangular matmuls, exactly the layout the
    indirect-DMA gather wants its per-partition row indices in.
  - rolled (up[t-1]) inside a tile is the gathered tile shifted down one
    partition. Compute engines cannot read partition-shifted operands
    (quadrant-aligned bases only), so the shift rides the PE: a matmul
    with a shifted-identity weight (bitwise exact on HW — the fp32
    LOW/HIGH weight split recomposes exactly for 0/1 weights).
  - rows t = 128k blend against the previous tile's last row, which the
    per-tile shift can't see. All 32 of them are redone in one batched
    epilogue: two 32-row gathers (z[idx[128k]], z[idx[128k-1]]), one ACT
    mul, one DVE blend, then one 16-row scatter store per HWDGE ring,
    each issued after that ring's main stores so FIFO order makes the
    overwrite win.
  - main stores alternate between the two HWDGE rings (one ring's
    descriptor generation caps at ~240 GB/s; the store-only drain at the
    tail needs both) and each trigger is issued 4 iterations late so its
    embedded wait-on-blend never head-of-line-blocks the ring's engine
    (the scalar ring shares its engine with the ACT t1 ops).
  - out[0] = up[0] exactly via forcing p[0] = 1 (q[0] = 0).
"""

import numpy as np

import concourse.bacc as bacc
import concourse.bass as bass
import concourse.mybir as mybir
import concourse.tile as tile
from concourse.bass import IndirectOffsetOnAxis
from concourse.bass_utils import run_bass_kernel_spmd

# Problem shape (hardcoded per harness contract).
B = 8          # batch rows == number of cores
T = 4096       # timesteps per row
NCH = 2048     # number of chunks (z rows)
D = 1024       # d_model
P = 128        # SBUF partitions
NT = T // P    # 32 tiles per core
NCOL = T // P  # 32 columns in the W layout
DH = D // 2    # matmul free-dim max for fp32 is 512

F32 = mybir.dt.float32
I32 = mybir.dt.int32

# hot consts layout (columns of a [128, 260] fp32 tensor): needed by the
# index-scan critical path.
H_TRI = 0      # [128, 128]  tri[k, i] = 1 iff i > k
H_ONEC = 128   # [128, 1]    ones column
H_W = 130      # ones row = row 0 of cols 1:129 (tri row 0 is ones there)
# cold consts layout ([128, 192]): needed a few us later.
C_ISH = 0      # [128, 128]  ish[k, i] = 1 iff i == k + 1
C_TRI32 = 128  # [32, 32]    tri32[k, j] = 1 iff j > k
C_ID32 = 160   # [32, 32]    identity
C_W = 192


def host_consts() -> tuple[np.ndarray, np.ndarray]:
    h = np.zeros((P, H_W), dtype=np.float32)
    h[:, H_TRI : H_TRI + P] = np.triu(np.ones((P, P), np.float32), 1)
    h[:, H_ONEC] = 1.0
    c = np.zeros((P, C_W), dtype=np.float32)
    c[:, C_ISH : C_ISH + P] = np.eye(P, P, 1, dtype=np.float32)
    c[:NCOL, C_TRI32 : C_TRI32 + NCOL] = np.triu(np.ones((NCOL, NCOL), np.float32), 1)
    c[:NCOL, C_ID32 : C_ID32 + NCOL] = np.eye(NCOL, dtype=np.float32)
    return h, c


def build_bass() -> bass.Bass:
    # Bacc (not raw Bass): its finalize() runs generate_event_semaphores,
    # which splits multi-sem waits to satisfy TRN2's one-wait-per-instruction
    # ISA constraint.
    nc = bacc.Bacc()

    z = nc.dram_tensor("z", [NCH, D], F32, kind="ExternalInput")
    p = nc.dram_tensor("p", [T], F32, kind="ExternalInput")
    b = nc.dram_tensor("b", [T], I32, kind="ExternalInput")
    ch = nc.dram_tensor("consts_hot", [P, H_W], F32, kind="ExternalInput")
    cc = nc.dram_tensor("consts_cold", [P, C_W], F32, kind="ExternalInput")
    out = nc.dram_tensor("out", [T, D], F32, kind="ExternalOutput")

    with tile.TileContext(nc) as tc:
        with (
            tc.tile_pool(name="setup", bufs=1) as sp,
            tc.tile_pool(name="psmall", bufs=2, space="PSUM") as pps,
            tc.tile_pool(name="proll", bufs=3, space="PSUM") as ppr,
            tc.tile_pool(name="gat", bufs=18) as gp,
            tc.tile_pool(name="mid", bufs=6) as tp,
            tc.tile_pool(name="outp", bufs=12) as op_,
        ):
            # ---- setup loads, interleaved across the two HWDGE rings -------
            # (each dma_start costs ~0.65 us on its issuing engine, so the
            # critical b0/tri loads go first on different rings)
            hot = sp.tile([P, H_W], F32)
            nc.scalar.dma_start(out=hot[:], in_=ch[:])
            tri = hot[:, H_TRI : H_TRI + P]
            ones_col = hot[:, H_ONEC : H_ONEC + 1]
            ones_row = hot[0:1, 1 : 1 + P]  # tri row 0 + ones_col head: all ones

            b_w_view = b[:].rearrange("(j q) -> q j", q=P)  # [128, 32] DRAM view
            b0_i = sp.tile([P, 1], I32)
            nc.sync.dma_start(out=b0_i[:], in_=b_w_view[:, 0:1])

            cold = sp.tile([P, C_W], F32)
            nc.scalar.dma_start(out=cold[:], in_=cc[:])
            ishift = cold[:, C_ISH : C_ISH + P]
            tri32 = cold[0:NCOL, C_TRI32 : C_TRI32 + NCOL]
            id32 = cold[0:NCOL, C_ID32 : C_ID32 + NCOL]

            b2d = b[:].rearrange("(j c) -> j c", c=P)  # [32, 128] DRAM view
            p2d = p[:].rearrange("(j c) -> j c", c=P)
            b_nat_i = sp.tile([NCOL, P], I32)
            nc.sync.dma_start(out=b_nat_i[:], in_=b2d)
            p_nat = sp.tile([NCOL, P], F32)
            nc.scalar.dma_start(out=p_nat[:], in_=p2d)

            # ---- tile-0 index short path -----------------------------------
            b0_f = sp.tile([P, 1], F32)
            nc.vector.tensor_copy(out=b0_f[:], in_=b0_i[:])
            s0_ps = pps.tile([P, 1], F32, space="PSUM", tag="small_ps")
            nc.tensor.matmul(out=s0_ps[:], lhsT=tri, rhs=b0_f[:],
                             start=True, stop=True)
            idx0_i = sp.tile([P, 1], I32)
            nc.vector.tensor_scalar_min(out=idx0_i[:], in0=s0_ps[:],
                                        scalar1=float(NCH - 1))
            # tile-0 gather issues here, ahead of the full index chain, so
            # nothing on the gpsimd queue delays the first HBM burst
            up0 = gp.tile([P, D], F32, tag="up")
            nc.gpsimd.indirect_dma_start(
                out=up0[:], out_offset=None, in_=z[:],
                in_offset=IndirectOffsetOnAxis(ap=idx0_i[:, 0:1], axis=0),
            )

            # ---- full index chain ------------------------------------------
            b_nat = sp.tile([NCOL, P], F32)
            nc.vector.tensor_copy(out=b_nat[:], in_=b_nat_i[:])
            bw_ps = pps.tile([P, NCOL], F32, space="PSUM", tag="small_ps")
            nc.tensor.transpose(out=bw_ps[:], in_=b_nat[:], identity=id32)
            b_w = sp.tile([P, NCOL], F32)
            nc.vector.tensor_copy(out=b_w[:], in_=bw_ps[:])

            # column sums via DVE free-axis reduction (exact: 0/1 summands),
            # in parallel with the PE transpose chain
            tot_col = sp.tile([NCOL, 1], F32)
            nc.vector.tensor_reduce(out=tot_col[:], in_=b_nat[:],
                                    axis=mybir.AxisListType.X,
                                    op=mybir.AluOpType.add)
            cofs_ps = pps.tile([1, NCOL], F32, space="PSUM", tag="small_ps")
            nc.tensor.matmul(out=cofs_ps[:], lhsT=tot_col[:], rhs=tri32,
                             start=True, stop=True)
            colofs = sp.tile([1, NCOL], F32)
            nc.vector.tensor_copy(out=colofs[:], in_=cofs_ps[:])

            s_ps = pps.tile([P, NCOL], F32, space="PSUM", tag="small_ps")
            nc.tensor.matmul(out=s_ps[:], lhsT=tri, rhs=b_w[:],
                             start=True, stop=False)
            nc.tensor.matmul(out=s_ps[:], lhsT=ones_row, rhs=colofs[:],
                             start=False, stop=True)
            idx_i = sp.tile([P, NCOL], I32)
            nc.vector.tensor_scalar_min(out=idx_i[:], in0=s_ps[:],
                                        scalar1=float(NCH - 1))
            idx_f = sp.tile([P, NCOL], F32)  # f32 copy for the epilogue rows
            nc.vector.tensor_scalar_min(out=idx_f[:], in0=s_ps[:],
                                        scalar1=float(NCH - 1))

            # ---- p / q in W layout -----------------------------------------
            pw_ps = pps.tile([P, NCOL], F32, space="PSUM", tag="small_ps")
            nc.tensor.transpose(out=pw_ps[:], in_=p_nat[:], identity=id32)
            p_w = sp.tile([P, NCOL], F32)
            nc.vector.tensor_copy(out=p_w[:], in_=pw_ps[:])
            # out[0] = up[0] exactly: force p[0] = 1 so the blend is 1*up + 0*rolled
            nc.vector.memset(p_w[0:1, 0:1], 1.0)
            q_w = sp.tile([P, NCOL], F32)  # q = 1 - p
            nc.scalar.activation(
                out=q_w[:], in_=p_w[:],
                func=mybir.ActivationFunctionType.Copy, bias=1.0, scale=-1.0,
            )

            # PE warm-up: the HAM clock gate keeps the PE at half clock until
            # it has accumulated a few us of busy time. These two matmuls sit
            # after the index chain in the PE queue (so they don't delay the
            # first gather) and finish right as tile 0's data lands, so the
            # main-loop shifts run at full clock from the start.
            for _ in range(4):
                wps = ppr.tile([P, C_W], F32, space="PSUM", tag="roll")
                nc.tensor.matmul(out=wps[:], lhsT=ishift, rhs=cold[:, 0:C_W],
                                 start=True, stop=True, skip_group_check=True)


            # ---- epilogue vectors for rows t = 128k ------------------------
            # bidx[k] = idx[128k - 1] (0 for k=0, harmless: q[0]=0). Row 127
            # of idx_f is not a legal compute-engine base, so extract it with
            # a tiny SBUF->SBUF DMA, then rotate rows into columns with
            # [1,32]-lhsT matmuls against a single 1.0.
            brow = sp.tile([1, NCOL], F32)
            nc.vector.memset(brow[:], 0.0)
            nc.sync.dma_start(
                out=brow[0:1, 1:NCOL], in_=idx_f[P - 1 : P, 0 : NCOL - 1]
            )
            cols_ps = pps.tile([NCOL, 4], F32, space="PSUM", tag="small_ps")
            for ci, row in enumerate([brow[0:1, 0:NCOL], idx_f[0:1, 0:NCOL],
                                      p_w[0:1, 0:NCOL], q_w[0:1, 0:NCOL]]):
                nc.tensor.matmul(
                    out=cols_ps[:, ci : ci + 1], lhsT=row,
                    rhs=ones_row[0:1, 0:1], start=True, stop=True,
                )
            bidx_i = sp.tile([NCOL, 1], I32)
            nc.vector.tensor_copy(out=bidx_i[:], in_=cols_ps[:, 0:1])
            fidx_i = sp.tile([NCOL, 1], I32)
            nc.vector.tensor_copy(out=fidx_i[:], in_=cols_ps[:, 1:2])
            pb_col = sp.tile([NCOL, 1], F32)
            nc.vector.tensor_copy(out=pb_col[:], in_=cols_ps[:, 2:3])
            qb_col = sp.tile([NCOL, 1], F32)
            nc.vector.tensor_copy(out=qb_col[:], in_=cols_ps[:, 3:4])

            # ---- main loop: gather, roll, blend, store ---------------------
            os_ = []
            for k in range(NT):
                if k == 0:
                    up = up0
                else:
                    up = gp.tile([P, D], F32, tag="up")
                    nc.gpsimd.indirect_dma_start(
                        out=up[:], out_offset=None, in_=z[:],
                        in_offset=IndirectOffsetOnAxis(ap=idx_i[:, k : k + 1], axis=0),
                    )

                if k == 8:
                    # epilogue gathers, slotted here on the gpsimd queue so
                    # they neither delay tiles 0-1 nor extend the tail
                    upf = sp.tile([NCOL, D], F32)
                    nc.gpsimd.indirect_dma_start(
                        out=upf[:], out_offset=None, in_=z[:],
                        in_offset=IndirectOffsetOnAxis(ap=fidx_i[:, 0:1], axis=0),
                    )
                    rollf = sp.tile([NCOL, D], F32)
                    nc.gpsimd.indirect_dma_start(
                        out=rollf[:], out_offset=None, in_=z[:],
                        in_offset=IndirectOffsetOnAxis(ap=bidx_i[:, 0:1], axis=0),
                    )


                # t1 = p * up on ACT
                t1 = tp.tile([P, D], F32, tag="t1")
                nc.scalar.mul(out=t1[:], in_=up[:], mul=p_w[:, k : k + 1])

                # rolled[i] = up[i-1] via PE shifted-identity matmul
                rps = ppr.tile([P, D], F32, space="PSUM", tag="roll")
                for h in range(2):
                    sl = slice(h * DH, (h + 1) * DH)
                    nc.tensor.matmul(out=rps[:, sl], lhsT=ishift, rhs=up[:, sl],
                                     start=True, stop=True, skip_group_check=True)

                # o = (rolled * q) + t1 on DVE, one op across both banks.
                # Row 0 of o comes out as t1[0] (rolled row 0 is zero); for
                # k = 0 that IS the right answer (q[0] = 0), for k > 0 it is
                # patched below from the previous tile's last row.
                # NOTE: stores must keep all 128 partitions — a 127-partition
                # pattern can't be split across the 16 DMA engines and
                # serializes onto one (measured 17x slower).
                o = op_.tile([P, D], F32, tag="o")
                nc.vector.scalar_tensor_tensor(
                    out=o[:], in0=rps[:], scalar=q_w[:, k : k + 1], in1=t1[:],
                    op0=mybir.AluOpType.mult, op1=mybir.AluOpType.add,
                )
                os_.append(o)
                # alternate stores across the two HWDGE rings (a single
                # queue's descriptor generation caps at ~240 GB/s, so the
                # store-only drain at the tail needs both), and issue each
                # trigger 3 iterations late so its embedded wait-on-blend is
                # already satisfied and never head-of-line-blocks the ring's
                # engine (the scalar ring shares its engine with the t1 ops)
                j = k - 4
                if j >= 0:
                    store_eng = nc.sync if j % 2 == 0 else nc.scalar
                    store_eng.dma_start(out=out[j * P : (j + 1) * P, :],
                                        in_=os_[j][:])

                if k == 10:
                    # epilogue blend for all rows t = 128k
                    t1b = sp.tile([NCOL, D], F32)
                    nc.scalar.mul(out=t1b[:], in_=upf[:], mul=pb_col[:])
                    ob = sp.tile([NCOL, D], F32)
                    nc.vector.scalar_tensor_tensor(
                        out=ob[:], in0=rollf[:], scalar=qb_col[:], in1=t1b[:],
                        op0=mybir.AluOpType.mult, op1=mybir.AluOpType.add,
                    )

            for j in range(NT - 4, NT):
                store_eng = nc.sync if j % 2 == 0 else nc.scalar
                store_eng.dma_start(out=out[j * P : (j + 1) * P, :],
                                    in_=os_[j][:])

            # epilogue scatter: redo rows t = 128k exactly. One scatter per
            # store queue, each after that queue's main stores, so FIFO order
            # makes the overwrite win on both rings.
            out_rows0 = out[:].rearrange("(j r) d -> j r d", r=P)
            nc.sync.dma_start(
                out=out_rows0[0:NCOL:2, 0:1, :], in_=ob[0:NCOL:2, None, :]
            )
            nc.scalar.dma_start(
                out=out_rows0[1:NCOL:2, 0:1, :], in_=ob[1:NCOL:2, None, :]
            )


    # Run the Bacc lowering passes (register allocation, event-semaphore
    # splitting, ...) — run_bass_via_pjrt serializes nc.m as-is.
    nc.finalize()
    return nc


_NC_CACHE = None


def _get_nc() -> bass.Bass:
    global _NC_CACHE
    if _NC_CACHE is None:
        _NC_CACHE = build_bass()
    return _NC_CACHE


def make_in_maps(z: np.ndarray, p: np.ndarray, b: np.ndarray) -> list[dict]:
    hot, cold = host_consts()
    return [
        {
            "z": np.ascontiguousarray(z[i], dtype=np.float32),
            "p": np.ascontiguousarray(p[i], dtype=np.float32),
            "b": np.ascontiguousarray(b[i], dtype=np.int32),
            "consts_hot": hot,
            "consts_cold": cold,
        }
        for i in range(B)
    ]


def kernel(z, p, b, original_len=None, **_unused) -> np.ndarray:
    z = np.asarray(z, dtype=np.float32)
    p = np.asarray(p, dtype=np.float32)
    b = np.asarray(b, dtype=np.int32)
    assert z.shape == (B, NCH, D) and p.shape == (B, T) and b.shape == (B, T)

    nc = _get_nc()
    res = run_bass_kernel_spmd(nc, make_in_maps(z, p, b), list(range(B)))
    return np.stack([r["out"] for r in res.results], axis=0)


# revision 31
# speedup vs baseline: 1.0165x; 1.0165x over previous
"""Trainium2 Bass kernel for nn_DechunkingLayer (ragged_sequence).

Reference semantics (per batch row):
    idx = clip(exclusive_cumsum(b), 0, NC - 1)          # [T]
    up[t]  = z[idx[t]]                                  # gather rows
    out[t] = p[t] * up[t] + (1 - p[t]) * up[t-1]        # EMA blend
    out[0] = up[0]

Sharding: pure data parallel over batch B=8 across the 8 NeuronCores
(one batch row per core). All work per row is independent.

Per-core plan. HBM traffic = 16 MB gather + 16 MB store = 32 MB and the
steady state runs at the HBM roofline (~410 GB/s combined), so the
optimization targets are the two ends of the timeline:
  - constants (triangular scan matrix, shifted identity, ...) come in as
    host-provided input tensors, split hot/cold across the two HWDGE
    rings so the scan matrix lands ~2 us after the NEFF preamble; no
    gpsimd constant building, no PE warm-up matmuls ahead of the scan.
  - tile-0 gather indices take a short path: b[0:128] is DMA'd as a
    [128, 1] column directly (colofs[0] = 0, so column 0 needs only the
    partition-dim triangular-matmul scan).
  - full cumsum in the [128, 32] "W layout" (partition = t % 128,
    column = t // 128) via PE triangular matmuls, exactly the layout the
    indirect-DMA gather wants its per-partition row indices in.
  - rolled (up[t-1]) inside a tile is the gathered tile shifted down one
    partition. Compute engines cannot read partition-shifted operands
    (quadrant-aligned bases only), so the shift rides the PE: a matmul
    with a shifted-identity weight (bitwise exact on HW — the fp32
    LOW/HIGH weight split recomposes exactly for 0/1 weights).
  - rows t = 128k blend against the previous tile's last row, which the
    per-tile shift can't see. All 32 of them are redone in one batched
    epilogue: two 32-row gathers (z[idx[128k]], z[idx[128k-1]]), one ACT
    mul, one DVE blend, then one 16-row scatter store per HWDGE ring,
    each issued after that ring's main stores so FIFO order makes the
    overwrite win.
  - main stores alternate between the two HWDGE rings (one ring's
    descriptor generation caps at ~240 GB/s; the store-only drain at the
    tail needs both) and each trigger is issued 4 iterations late so its
    embedded wait-on-blend never head-of-line-blocks the ring's engine
    (the scalar ring shares its engine with the ACT t1 ops).
  - out[0] = up[0] exactly via forcing p[0] = 1 (q[0] = 0).
"""

import numpy as np

import concourse.bacc as bacc
import concourse.bass as bass
import concourse.mybir as mybir
import concourse.tile as tile
from concourse.bass import IndirectOffsetOnAxis
from concourse.bass_utils import run_bass_kernel_spmd

# Problem shape (hardcoded per harness contract).
B = 8          # batch rows == number of cores
T = 4096       # timesteps per row
NCH = 2048     # number of chunks (z rows)
D = 1024       # d_model
P = 128        # SBUF partitions
NT = T // P    # 32 tiles per core
NCOL = T // P  # 32 columns in the W layout
DH = D // 2    # matmul free-dim max for fp32 is 512

F32 = mybir.dt.float32
I32 = mybir.dt.int32

# hot consts layout (columns of a [128, 260] fp32 tensor): needed by the
# index-scan critical path.
H_TRI = 0      # [128, 128]  tri[k, i] = 1 iff i > k
H_ONEC = 128   # [128, 1]    ones column
H_W = 130      # ones row = row 0 of cols 1:129 (tri row 0 is ones there)
# cold consts layout ([128, 192]): needed a few us later.
C_ISH = 0      # [128, 128]  ish[k, i] = 1 iff i == k + 1
C_TRI32 = 128  # [32, 32]    tri32[k, j] = 1 iff j > k
C_ID32 = 160   # [32, 32]    identity
C_W = 192


def host_consts() -> tuple[np.ndarray, np.ndarray]:
    h = np.zeros((P, H_W), dtype=np.float32)
    h[:, H_TRI : H_TRI + P] = np.triu(np.ones((P, P), np.float32), 1)
    h[:, H_ONEC] = 1.0
    c = np.zeros((P, C_W), dtype=np.float32)
    c[:, C_ISH : C_ISH + P] = np.eye(P, P, 1, dtype=np.float32)
    c[:NCOL, C_TRI32 : C_TRI32 + NCOL] = np.triu(np.ones((NCOL, NCOL), np.float32), 1)
    c[:NCOL, C_ID32 : C_ID32 + NCOL] = np.eye(NCOL, dtype=np.float32)
    return h, c


def build_bass() -> bass.Bass:
    # Bacc (not raw Bass): its finalize() runs generate_event_semaphores,
    # which splits multi-sem waits to satisfy TRN2's one-wait-per-instruction
    # ISA constraint.
    nc = bacc.Bacc()

    z = nc.dram_tensor("z", [NCH, D], F32, kind="ExternalInput")
    p = nc.dram_tensor("p", [T], F32, kind="ExternalInput")
    b = nc.dram_tensor("b", [T], I32, kind="ExternalInput")
    ch = nc.dram_tensor("consts_hot", [P, H_W], F32, kind="ExternalInput")
    cc = nc.dram_tensor("consts_cold", [P, C_W], F32, kind="ExternalInput")
    out = nc.dram_tensor("out", [T, D], F32, kind="ExternalOutput")

    with tile.TileContext(nc) as tc:
        with (
            tc.tile_pool(name="setup", bufs=1) as sp,
            tc.tile_pool(name="psmall", bufs=2, space="PSUM") as pps,
            tc.tile_pool(name="proll", bufs=3, space="PSUM") as ppr,
            tc.tile_pool(name="gat", bufs=18) as gp,
            tc.tile_pool(name="mid", bufs=6) as tp,
            tc.tile_pool(name="outp", bufs=12) as op_,
        ):
            # ---- setup loads, interleaved across the two HWDGE rings -------
            # (each dma_start costs ~0.65 us on its issuing engine, so the
            # critical b0/tri loads go first on different rings)
            hot = sp.tile([P, H_W], F32)
            nc.scalar.dma_start(out=hot[:], in_=ch[:])
            tri = hot[:, H_TRI : H_TRI + P]
            ones_col = hot[:, H_ONEC : H_ONEC + 1]
            ones_row = hot[0:1, 1 : 1 + P]  # tri row 0 + ones_col head: all ones

            b_w_view = b[:].rearrange("(j q) -> q j", q=P)  # [128, 32] DRAM view
            b0_i = sp.tile([P, 1], I32)
            nc.sync.dma_start(out=b0_i[:], in_=b_w_view[:, 0:1])

            cold = sp.tile([P, C_W], F32)
            nc.scalar.dma_start(out=cold[:], in_=cc[:])
            ishift = cold[:, C_ISH : C_ISH + P]
            tri32 = cold[0:NCOL, C_TRI32 : C_TRI32 + NCOL]
            id32 = cold[0:NCOL, C_ID32 : C_ID32 + NCOL]

            b2d = b[:].rearrange("(j c) -> j c", c=P)  # [32, 128] DRAM view
            p2d = p[:].rearrange("(j c) -> j c", c=P)
            b_nat_i = sp.tile([NCOL, P], I32)
            nc.sync.dma_start(out=b_nat_i[:], in_=b2d)
            p_nat = sp.tile([NCOL, P], F32)
            nc.scalar.dma_start(out=p_nat[:], in_=p2d)

            # ---- tile-0 index short path -----------------------------------
            b0_f = sp.tile([P, 1], F32)
            nc.vector.tensor_copy(out=b0_f[:], in_=b0_i[:])
            s0_ps = pps.tile([P, 1], F32, space="PSUM", tag="small_ps")
            nc.tensor.matmul(out=s0_ps[:], lhsT=tri, rhs=b0_f[:],
                             start=True, stop=True)
            idx0_i = sp.tile([P, 1], I32)
            nc.vector.tensor_scalar_min(out=idx0_i[:], in0=s0_ps[:],
                                        scalar1=float(NCH - 1))
            # tile-0 gather issues here, ahead of the full index chain, so
            # nothing on the gpsimd queue delays the first HBM burst
            up0 = gp.tile([P, D], F32, tag="up")
            nc.gpsimd.indirect_dma_start(
                out=up0[:], out_offset=None, in_=z[:],
                in_offset=IndirectOffsetOnAxis(ap=idx0_i[:, 0:1], axis=0),
            )

            # ---- full index chain ------------------------------------------
            b_nat = sp.tile([NCOL, P], F32)
            nc.vector.tensor_copy(out=b_nat[:], in_=b_nat_i[:])
            bw_ps = pps.tile([P, NCOL], F32, space="PSUM", tag="small_ps")
            nc.tensor.transpose(out=bw_ps[:], in_=b_nat[:], identity=id32)
            b_w = sp.tile([P, NCOL], F32)
            nc.vector.tensor_copy(out=b_w[:], in_=bw_ps[:])

            # column sums via DVE free-axis reduction (exact: 0/1 summands),
            # in parallel with the PE transpose chain
            tot_col = sp.tile([NCOL, 1], F32)
            nc.vector.tensor_reduce(out=tot_col[:], in_=b_nat[:],
                                    axis=mybir.AxisListType.X,
                                    op=mybir.AluOpType.add)
            cofs_ps = pps.tile([1, NCOL], F32, space="PSUM", tag="small_ps")
            nc.tensor.matmul(out=cofs_ps[:], lhsT=tot_col[:], rhs=tri32,
                             start=True, stop=True)
            colofs = sp.tile([1, NCOL], F32)
            nc.vector.tensor_copy(out=colofs[:], in_=cofs_ps[:])

            s_ps = pps.tile([P, NCOL], F32, space="PSUM", tag="small_ps")
            nc.tensor.matmul(out=s_ps[:], lhsT=tri, rhs=b_w[:],
                             start=True, stop=False)
            nc.tensor.matmul(out=s_ps[:], lhsT=ones_row, rhs=colofs[:],
                             start=False, stop=True)
            idx_i = sp.tile([P, NCOL], I32)
            nc.vector.tensor_scalar_min(out=idx_i[:], in0=s_ps[:],
                                        scalar1=float(NCH - 1))
            idx_f = sp.tile([P, NCOL], F32)  # f32 copy for the epilogue rows
            nc.vector.tensor_scalar_min(out=idx_f[:], in0=s_ps[:],
                                        scalar1=float(NCH - 1))

            # ---- p / q in W layout -----------------------------------------
            pw_ps = pps.tile([P, NCOL], F32, space="PSUM", tag="small_ps")
            nc.tensor.transpose(out=pw_ps[:], in_=p_nat[:], identity=id32)
            p_w = sp.tile([P, NCOL], F32)
            nc.vector.tensor_copy(out=p_w[:], in_=pw_ps[:])
            # out[0] = up[0] exactly: force p[0] = 1 so the blend is 1*up + 0*rolled
            nc.vector.memset(p_w[0:1, 0:1], 1.0)
            q_w = sp.tile([P, NCOL], F32)  # q = 1 - p
            nc.scalar.activation(
                out=q_w[:], in_=p_w[:],
                func=mybir.ActivationFunctionType.Copy, bias=1.0, scale=-1.0,
            )

            # PE warm-up: the HAM clock gate keeps the PE at half clock until
            # it has accumulated a few us of busy time. These two matmuls sit
            # after the index chain in the PE queue (so they don't delay the
            # first gather) and finish right as tile 0's data lands, so the
            # main-loop shifts run at full clock from the start.
            for _ in range(4):
                wps = ppr.tile([P, C_W], F32, space="PSUM", tag="roll")
                nc.tensor.matmul(out=wps[:], lhsT=ishift, rhs=cold[:, 0:C_W],
                                 start=True, stop=True, skip_group_check=True)


            # ---- epilogue vectors for rows t = 128k ------------------------
            # bidx[k] = idx[128k - 1] (0 for k=0, harmless: q[0]=0). Row 127
            # of idx_f is not a legal compute-engine base, so extract it with
            # a tiny SBUF->SBUF DMA, then rotate rows into columns with
            # [1,32]-lhsT matmuls against a single 1.0.
            brow = sp.tile([1, NCOL], F32)
            nc.vector.memset(brow[:], 0.0)
            nc.sync.dma_start(
                out=brow[0:1, 1:NCOL], in_=idx_f[P - 1 : P, 0 : NCOL - 1]
            )
            cols_ps = pps.tile([NCOL, 4], F32, space="PSUM", tag="small_ps")
            for ci, row in enumerate([brow[0:1, 0:NCOL], idx_f[0:1, 0:NCOL],
                                      p_w[0:1, 0:NCOL], q_w[0:1, 0:NCOL]]):
                nc.tensor.matmul(
                    out=cols_ps[:, ci : ci + 1], lhsT=row,
                    rhs=ones_row[0:1, 0:1], start=True, stop=True,
                )
            bidx_i = sp.tile([NCOL, 1], I32)
            nc.vector.tensor_copy(out=bidx_i[:], in_=cols_ps[:, 0:1])
            fidx_i = sp.tile([NCOL, 1], I32)
            nc.vector.tensor_copy(out=fidx_i[:], in_=cols_ps[:, 1:2])
            pb_col = sp.tile([NCOL, 1], F32)
            nc.vector.tensor_copy(out=pb_col[:], in_=cols_ps[:, 2:3])
            qb_col = sp.tile([NCOL, 1], F32)
            nc.vector.tensor_copy(out=qb_col[:], in_=cols_ps[:, 3:4])

            # ---- main loop: gather, roll, blend, store ---------------------
            os_ = []
            for k in range(NT):
                if k == 0:
                    up = up0
                else:
                    up = gp.tile([P, D], F32, tag="up")
                    nc.gpsimd.indirect_dma_start(
                        out=up[:], out_offset=None, in_=z[:],
                        in_offset=IndirectOffsetOnAxis(ap=idx_i[:, k : k + 1], axis=0),
                    )

                if k == 20:
                    # epilogue gathers, slotted mid-kernel on the gpsimd queue
                    # where it has ~1 us/tile of slack — at ramp positions
                    # (k < ~16) the queue is trigger-rate-limited and these
                    # two extra triggers would delay the whole gather stream
                    upf = sp.tile([NCOL, D], F32)
                    nc.gpsimd.indirect_dma_start(
                        out=upf[:], out_offset=None, in_=z[:],
                        in_offset=IndirectOffsetOnAxis(ap=fidx_i[:, 0:1], axis=0),
                    )
                    rollf = sp.tile([NCOL, D], F32)
                    nc.gpsimd.indirect_dma_start(
                        out=rollf[:], out_offset=None, in_=z[:],
                        in_offset=IndirectOffsetOnAxis(ap=bidx_i[:, 0:1], axis=0),
                    )


                # t1 = p * up on ACT
                t1 = tp.tile([P, D], F32, tag="t1")
                nc.scalar.mul(out=t1[:], in_=up[:], mul=p_w[:, k : k + 1])

                # rolled[i] = up[i-1] via PE shifted-identity matmul
                rps = ppr.tile([P, D], F32, space="PSUM", tag="roll")
                for h in range(2):
                    sl = slice(h * DH, (h + 1) * DH)
                    nc.tensor.matmul(out=rps[:, sl], lhsT=ishift, rhs=up[:, sl],
                                     start=True, stop=True, skip_group_check=True)

                # o = (rolled * q) + t1 on DVE, one op across both banks.
                # Row 0 of o comes out as t1[0] (rolled row 0 is zero); for
                # k = 0 that IS the right answer (q[0] = 0), for k > 0 it is
                # patched below from the previous tile's last row.
                # NOTE: stores must keep all 128 partitions — a 127-partition
                # pattern can't be split across the 16 DMA engines and
                # serializes onto one (measured 17x slower).
                o = op_.tile([P, D], F32, tag="o")
                nc.vector.scalar_tensor_tensor(
                    out=o[:], in0=rps[:], scalar=q_w[:, k : k + 1], in1=t1[:],
                    op0=mybir.AluOpType.mult, op1=mybir.AluOpType.add,
                )
                os_.append(o)
                # alternate stores across the two HWDGE rings (a single
                # queue's descriptor generation caps at ~240 GB/s, so the
                # store-only drain at the tail needs both), and issue each
                # trigger 3 iterations late so its embedded wait-on-blend is
                # already satisfied and never head-of-line-blocks the ring's
                # engine (the scalar ring shares its engine with the t1 ops)
                j = k - 4
                if j >= 0:
                    store_eng = nc.sync if j % 2 == 0 else nc.scalar
                    store_eng.dma_start(out=out[j * P : (j + 1) * P, :],
                                        in_=os_[j][:])

                if k == 22:
                    # epilogue blend for all rows t = 128k
                    t1b = sp.tile([NCOL, D], F32)
                    nc.scalar.mul(out=t1b[:], in_=upf[:], mul=pb_col[:])
                    ob = sp.tile([NCOL, D], F32)
                    nc.vector.scalar_tensor_tensor(
                        out=ob[:], in0=rollf[:], scalar=qb_col[:], in1=t1b[:],
                        op0=mybir.AluOpType.mult, op1=mybir.AluOpType.add,
                    )

            for j in range(NT - 4, NT):
                store_eng = nc.sync if j % 2 == 0 else nc.scalar
                store_eng.dma_start(out=out[j * P : (j + 1) * P, :],
                                    in_=os_[j][:])

            # epilogue scatter: redo rows t = 128k exactly. One scatter per
            # store queue, each after that queue's main stores, so FIFO order
            # makes the overwrite win on both rings.
            out_rows0 = out[:].rearrange("(j r) d -> j r d", r=P)
            nc.sync.dma_start(
                out=out_rows0[0:NCOL:2, 0:1, :], in_=ob[0:NCOL:2, None, :]
            )
            nc.scalar.dma_start(
                out=out_rows0[1:NCOL:2, 0:1, :], in_=ob[1:NCOL:2, None, :]
            )


    # Run the Bacc lowering passes (register allocation, event-semaphore
    # splitting, ...) — run_bass_via_pjrt serializes nc.m as-is.
    nc.finalize()
    return nc


_NC_CACHE = None


def _get_nc() -> bass.Bass:
    global _NC_CACHE
    if _NC_CACHE is None:
        _NC_CACHE = build_bass()
    return _NC_CACHE


def make_in_maps(z: np.ndarray, p: np.ndarray, b: np.ndarray) -> list[dict]:
    hot, cold = host_consts()
    return [
        {
            "z": np.ascontiguousarray(z[i], dtype=np.float32),
            "p": np.ascontiguousarray(p[i], dtype=np.float32),
            "b": np.ascontiguousarray(b[i], dtype=np.int32),
            "consts_hot": hot,
            "consts_cold": cold,
        }
        for i in range(B)
    ]


def kernel(z, p, b, original_len=None, **_unused) -> np.ndarray:
    z = np.asarray(z, dtype=np.float32)
    p = np.asarray(p, dtype=np.float32)
    b = np.asarray(b, dtype=np.int32)
    assert z.shape == (B, NCH, D) and p.shape == (B, T) and b.shape == (B, T)

    nc = _get_nc()
    res = run_bass_kernel_spmd(nc, make_in_maps(z, p, b), list(range(B)))
    return np.stack([r["out"] for r in res.results], axis=0)
